# revision 1
# baseline (speedup 1.0000x reference)
"""DRConv2d Trainium2 kernel — batch-parallel over 8 NeuronCores.

Per core (one sample b): x_b [64, 126, 126] -> out_b [64, 124, 124]

Math (per sample):
  pooled = avgpool3x3(x); s1 = sigmoid(w1 @ pooled + b1)
  kern[g] = w2[g] @ s1[g] + b2[g]          -> per-sample filter bank [8*64, 64, 3, 3]
  out_r = conv3x3_valid(x, kern)           -> [8, 64, 124, 124]
  guide = conv3x3_valid(x, wg) + bg        -> [8, 124, 124]
  out = out_{argmax_r guide}               -> [64, 124, 124]

Device strategy (cost-model-aware: matmul cost = out_free_size x dtype_rate,
independent of K and M; fp8 DoubleRow = 0.5x):
  - main conv fp16: x as two partition-stacked shifted layouts
    xa16=[v@0;v@+1], x216=[v@0;v@+126]; 9-tap conv = 4 paired K=128 matmuls
    per 128-channel chunk + 1 combined matmul (see below).
  - guide conv: fp16 main pass (5 matmuls) + compensated corrections
    (wl@x + wh@xl) in fp8 DoubleRow (6 matmuls at 0.5 = 3 slots), keeping
    argmax flips at the fp16x2-compensated level.
  - region one-hot mask: gpsimd partition all-reduce max + DVE is_equal.
  - selection: BIG*mask injected into each conv psum chunk through the
    combined matmul (tap-8 weights rows 0:64 + BIG*E_c rows 64:72, rhs is a
    staged [72,TN] tile holding the tap-8 x window + mask rows), then
    u_c = relu(P_c - BIG/2) and a DVE add-tree; no emask/fold matmuls.
  - one interleaved loop over 31 position tiles (4 output rows each).
"""
import numpy as np
import ml_dtypes

import concourse.bass as bass
import concourse.mybir as mybir
import concourse.tile as tile
from concourse import bacc, bass_isa, library_config
from concourse.bass_utils import run_bass_kernel_spmd

F32 = mybir.dt.float32
F16 = mybir.dt.float16
FP8 = mybir.dt.float8e4
FP8E5 = mybir.dt.float8e5
AL = mybir.AluOpType
AFT = mybir.ActivationFunctionType
DR = mybir.MatmulPerfMode.DoubleRow

R, CIN, COUT = 8, 64, 64
H, W = 126, 126
HO, WO = 124, 124
NPOS = HO * WO          # 15376
HW2 = H * W             # 15876
NT = 31                 # position tiles, 4 output rows each
TN = 4 * WO             # 496 positions per tile
G = 14                  # guide tiles run ahead of conv
GPRE = 6                # guide tiles before the generator block
BIG = 64.0              # mask inject magnitude; relu offset is BIG/2
CSC = 2.0 ** -8         # guide correction scale (host pre-scales by 2^8)

# conv/guide tap pairing: kern rows are (g, slot), pair taps at adjacent
# slots: slot -> hw: (0,1),(3,4),(6,7),(2,5) pairs + tap8 at slot 8
SLOT_HW = (0, 1, 3, 4, 6, 7, 2, 5, 8)
CPAIRS = [("xa16", 0, 0), ("xa16", 1, 0), ("xa16", 2, 0), ("x216", 0, 2)]
GMAIN = [("xa16", 0, 0, 128), ("xa16", 1, 0, 128), ("xa16", 2, 0, 128),
         ("x216", 0, 2, 128), ("x216", 2, 2, 64)]
# guide corr DR rhs windows: (ktile_stride, dy, dx)
CORR_SPECS = [(W, 0, 0), (2, 2, 0), (W, 0, 2)]


def _win(xv, t, dy, dx, k=128):
    r0 = 4 * t
    return xv[0:k, r0 + dy: r0 + dy + 4, dx: dx + WO]


def _corr_ap(xt, t, ktile_stride, dy, dx):
    """Manual DR rhs AP: [partition 128][ktile 2][row 4][col 124] with
    overlapping strides on the flat [128, HW2] x tile."""
    base = xt[:]
    off = base.offset + (4 * t + dy) * W + dx
    return bass.AP(base.tensor, off, [[HW2, 128], [ktile_stride, 2],
                                      [W, 4], [1, WO]])


def build_module():
    nc = bacc.Bacc(trn_type="TRN2", target_bir_lowering=False, debug=False,
                   num_devices=8)

    d_xs = {}
    for n, dt in (("xa16", F16), ("x216", F16), ("xa8", FP8), ("gxa8", FP8E5)):
        d_xs[n] = nc.dram_tensor(n, (128, HW2), dt, kind="ExternalInput")
    d_w1aug = nc.dram_tensor("w1aug", (65, 64), F32, kind="ExternalInput")
    d_krhs = nc.dram_tensor("krhs", (72, 4096), F16, kind="ExternalInput")
    d_sdelta = nc.dram_tensor("sdelta", (8, 72), F16, kind="ExternalInput")
    d_wgp16 = nc.dram_tensor("wgp16", (128, 5, 8), F16, kind="ExternalInput")
    d_wl = nc.dram_tensor("wl8", (128, 3, 2, 16), FP8E5, kind="ExternalInput")
    d_wh = nc.dram_tensor("wh8", (128, 3, 2, 16), FP8, kind="ExternalInput")
    d_bg = nc.dram_tensor("bg8", (8, 1), F32, kind="ExternalInput")
    d_ec = nc.dram_tensor("ec16", (8, 4, 128), F16, kind="ExternalInput")
    d_ones = nc.dram_tensor("ones64", (128, 64), F16, kind="ExternalInput")
    d_y = nc.dram_tensor("y", (COUT, NPOS), F32, kind="ExternalOutput")

    with tile.TileContext(nc) as tc:
        kernel_body(nc, tc, d_xs, d_w1aug, d_krhs, d_sdelta, d_wgp16,
                    d_wl, d_wh, d_bg, d_ec, d_ones, d_y)
    nc.compile()
    return nc


def kernel_body(nc, tc, d_xs, d_w1aug, d_krhs, d_sdelta, d_wgp16,
                d_wl, d_wh, d_bg, d_ec, d_ones, d_y):
    nc.gpsimd.load_library(library_config.mlp)

    with (
        tc.tile_pool(name="sbx", bufs=1) as sbx,
        tc.tile_pool(name="sbw", bufs=1) as sbw,
        tc.tile_pool(name="sbk", bufs=1) as sbk,
        tc.tile_pool(name="sbg", bufs=3) as sbg,
        tc.tile_pool(name="sst", bufs=G + 3) as sst,
        tc.tile_pool(name="sbu", bufs=2) as sbu,
        tc.tile_pool(name="sbo", bufs=3) as sbo,
        tc.tile_pool(name="p1", bufs=5, space="PSUM") as p1,   # conv chunks
        tc.tile_pool(name="p2", bufs=2, space="PSUM") as p2,   # guide
        tc.tile_pool(name="p3", bufs=1, space="PSUM") as p3,   # generator
    ):
        # ---- small weights on scalar ring; bulk on sync/scalar/pool ----
        wgp16 = sbw.tile([128, 5, 8], F16, tag="wgp16")
        nc.scalar.dma_start(wgp16[:], d_wgp16.ap())
        wl8, wh8 = [], []
        for m in range(3):
            tl = sbw.tile([128, 2, 16], FP8E5, tag=f"wl8_{m}")
            nc.scalar.dma_start(tl[:], d_wl.ap()[:, m, :, :])
            wl8.append(tl)
            th = sbw.tile([128, 2, 16], FP8, tag=f"wh8_{m}")
            nc.scalar.dma_start(th[:], d_wh.ap()[:, m, :, :])
            wh8.append(th)
        bg8 = sbw.tile([8, 1], F32, tag="bg8")
        nc.scalar.dma_start(bg8[:], d_bg.ap())

        krhs = sbk.tile([72, 4096], F16, tag="krhs")
        nc.sync.dma_start(krhs[:], d_krhs.ap())
        w1aug = sbw.tile([65, 64], F32, tag="w1aug")
        nc.sync.dma_start(w1aug[:], d_w1aug.ap())
        sdelta_t = sbw.tile([8, 72], F16, tag="sdelta")
        nc.sync.dma_start(sdelta_t[:], d_sdelta.ap())

        nb32 = sbw.tile([128, 1], F32, tag="nb32")
        nc.vector.memset(nb32[:], -BIG / 2)

        # x layouts, quarter-granular DMAs (fp16 on sync/scalar, fp8 on pool)
        xt = {}
        NQ = 8
        Q = (HW2 + NQ - 1) // NQ
        for n, dt in (("xa16", F16), ("x216", F16), ("xa8", FP8),
                      ("gxa8", FP8E5)):
            xt[n] = sbx.tile([128, HW2], dt, tag=n, name=n)
        for qi in range(NQ):
            lo, hi = Q * qi, min(Q * (qi + 1), HW2)
            nc.sync.dma_start(xt["xa16"][:, lo:hi], d_xs["xa16"].ap()[:, lo:hi])
            nc.scalar.dma_start(xt["x216"][:, lo:hi],
                                d_xs["x216"].ap()[:, lo:hi])
            for n in ("xa8", "gxa8"):
                nc.gpsimd.dma_start(xt[n][:, lo:hi], d_xs[n].ap()[:, lo:hi])
        xv16 = {n: xt[n][:].rearrange("p (h w) -> p h w", h=H)
                for n in ("xa16", "x216")}

        LW = sbw.tile([128, 16, 128], F16, tag="LW")
        LWE = sbw.tile([72, 4, 128], F16, tag="LWE")
        # mask-inject rows of LWE: BIG at (2c -> cols 0:64), (2c+1 -> 64:128)
        nc.scalar.dma_start(LWE[64:72, :, :], d_ec.ap())
        ones = sbw.tile([128, 64], F16, tag="ones")
        nc.sync.dma_start(ones[:], d_ones.ap())

        sts = {}

        def guide_tile(t):
            pg = p2.tile([40, TN], F32, tag="pg", name=f"pg{t}")
            for s, (xn, dy, dx, k) in enumerate(GMAIN):
                nc.tensor.matmul(pg[32:40, :], wgp16[0:k, s, :],
                                 _win(xv16[xn], t, dy, dx, k),
                                 start=(s == 0), stop=(s == 4),
                                 skip_group_check=True)
            # corr: pass A (wl*2^8 e5m2 @ xa8) + pass B (wh e4m3 @ gxa8)
            # mm0: ktiles dy0,dy1 -> taps 0,1,3,4
            # mm1: ktiles (dy2,dx0),(dy2,dx2) -> taps 6,7 + 8 (half zeroed)
            # mm2: ktiles (dy0,dx2),(dy1,dx2) -> taps 2,5 on top halves
            i = 0
            for wt, xn in ((wl8, "xa8"), (wh8, "gxa8")):
                for m, (ks, dy, dx) in enumerate(CORR_SPECS):
                    nc.tensor.matmul(pg[0:16, :], wt[m][:],
                                     _corr_ap(xt[xn], t, ks, dy, dx),
                                     start=(i == 0), stop=(i == 5),
                                     perf_mode=DR, skip_group_check=True)
                    i += 1
            gs = sbg.tile([8, TN], F32, tag="gs", name=f"gs{t}")
            nc.scalar.activation(gs[:], pg[32:40, :], AFT.Identity, bias=bg8[:])
            g = sbg.tile([8, TN], F32, tag="g", name=f"g{t}")
            nc.vector.scalar_tensor_tensor(g[:], pg[0:8, :], CSC, gs[:],
                                           op0=AL.mult, op1=AL.add)
            gm = sbg.tile([8, TN], F32, tag="gm", name=f"gm{t}")
            nc.gpsimd.partition_all_reduce(gm[:], g[:], channels=8,
                                           reduce_op=bass_isa.ReduceOp.max)
            st = sst.tile([72, TN], F16, tag="st", name=f"st{t}")
            stv = st[:].rearrange("p (r c) -> p r c", r=4)
            nc.scalar.copy(stv[0:64, :, :], _win(xv16["x216"], t, 2, 2, 64))
            nc.vector.tensor_tensor(st[64:72, :], g[:], gm[:], op=AL.is_equal)
            sts[t] = st

        # pooled rowsum reduce (DVE) overlaps the guide prologue on PE
        rs = sbk.tile([64, 378], F16, tag="rs")
        xrow = xt["xa16"][0:64, :].rearrange("p (a b) -> p a b", b=42)
        with nc.allow_low_precision(reason="pooled feeds sigmoid; fp16 ok"):
            for r0 in range(0, 378, 48):
                r1 = min(r0 + 48, 378)
                nc.vector.tensor_reduce(
                    rs[:, r0:r1], xrow[:, r0:r1, :],
                    axis=mybir.AxisListType.X, op=AL.add)
        # pooled cols in slot order: (0,1),(3,4),(6,7) at 0..5; 2,5,8 at 6..8
        pooled = sbg.tile([65, 9], F32, tag="pooled")
        rsv = rs[:].rearrange("p (kh r kw) -> p kh kw r", kh=3, r=42, kw=3)
        for kh in range(3):
            nc.vector.tensor_reduce(
                pooled[0:64, 2 * kh: 2 * kh + 2], rsv[:, kh, 0:2, :],
                axis=mybir.AxisListType.X, op=AL.add)
            nc.vector.tensor_reduce(
                pooled[0:64, 6 + kh: 7 + kh], rsv[:, kh, 2:3, :],
                axis=mybir.AxisListType.X, op=AL.add)
        nc.vector.memset(pooled[64:65, :], 1.0)

        for t in range(GPRE):
            guide_tile(t)

        # ---- generator: s1 -> kern (fp16) -> LW/LWE via pool-ring DMAs ----
        s1p = p3.tile([64, 9], F32, tag="pk", name="s1p")
        nc.tensor.matmul(s1p[:], w1aug[:], pooled[:], start=True, stop=True)
        s1s = sbg.tile([64, 9], F16, tag="s1s")
        nc.scalar.activation(s1s[:], s1p[:], AFT.Sigmoid)

        S = sbk.tile([72, 72], F16, tag="S")
        nc.vector.memset(S[:], 0.0)
        nc.scalar.copy(S[64:72, :], sdelta_t[:])
        for gi in range(8):
            nc.sync.dma_start(S[8 * gi:8 * gi + 8, 9 * gi:9 * gi + 9],
                              s1s[8 * gi:8 * gi + 8, :])

        ksb = sbk.tile([72, 4096], F16, tag="ksb")
        for j in range(8):
            pk = p3.tile([72, 512], F32, tag="pk", name=f"pk{j}")
            nc.tensor.matmul(pk[:], S[:], krhs[:, 512 * j: 512 * (j + 1)],
                             start=True, stop=True)
            nc.scalar.copy(ksb[:, 512 * j: 512 * (j + 1)], pk[:])

        # LW[s*64+ci, c*4+p, h*64+co] <- ksb[g*9+2p+s, ci*64+co], g = 2c+h
        ksv = ksb[:].rearrange("p (ci co) -> p ci co", ci=64)
        for c in range(4):
            for p in range(4):
                for hh in range(2):
                    gg = 2 * c + hh
                    nc.gpsimd.dma_start(
                        LW[0:128, c * 4 + p, 64 * hh:64 * hh + 64],
                        ksv[gg * 9 + 2 * p: gg * 9 + 2 * p + 2, :, :])
            for hh in range(2):
                gg = 2 * c + hh
                nc.gpsimd.dma_start(
                    LWE[0:64, c, 64 * hh:64 * hh + 64],
                    ksv[gg * 9 + 8: gg * 9 + 9, :, :])

        for t in range(GPRE, G):
            guide_tile(t)

        # ---- main loop: guide(t+G) + conv(t), relu-select, add-tree ----
        def conv_tile(t):
            st = sts.pop(t)
            pcs = []
            for c in range(4):
                pc = p1.tile([128, TN], F32, tag="pc", name=f"pc{c}_{t}")
                for p, (xn, dy, dx) in enumerate(CPAIRS):
                    nc.tensor.matmul(pc[:], LW[0:128, 4 * c + p, :],
                                     _win(xv16[xn], t, dy, dx, 128),
                                     start=(p == 0), stop=False)
                nc.tensor.matmul(pc[:], LWE[0:72, c, :], st[:],
                                 start=False, stop=True)
                pcs.append(pc)
            us = []
            for c in range(4):
                u = sbu.tile([128, TN], F16, tag=f"u{c}", name=f"u{c}_{t}")
                if c < 3:
                    nc.scalar.activation(u[:], pcs[c][:], AFT.Relu,
                                         bias=nb32[:])
                else:
                    nc.vector.tensor_scalar(u[:], pcs[c][:], -BIG / 2, 0.0,
                                            op0=AL.add, op1=AL.max)
                us.append(u)
            v0 = sbu.tile([128, TN], F16, tag="v0", name=f"v0_{t}")
            nc.vector.tensor_tensor(v0[:], us[0][:], us[1][:], op=AL.add)
            v1 = sbu.tile([128, TN], F16, tag="v1", name=f"v1_{t}")
            nc.vector.tensor_tensor(v1[:], us[2][:], us[3][:], op=AL.add)
            wv = sbu.tile([128, TN], F16, tag="wv", name=f"wv_{t}")
            nc.vector.tensor_tensor(wv[:], v0[:], v1[:], op=AL.add)
            pf = p3.tile([64, TN], F32, tag="pk", name=f"pf{t}")
            nc.tensor.matmul(pf[:], ones[:], wv[:], start=True, stop=True)
            ot = sbo.tile([64, TN], F32, tag="ot", name=f"ot{t}")
            nc.scalar.activation(ot[:], pf[:], AFT.Copy, bias=-BIG / 2)
            nc.sync.dma_start(d_y.ap()[:, TN * t: TN * (t + 1)], ot[:])

        for t in range(NT):
            if t + G < NT:
                guide_tile(t + G)
            conv_tile(t)


def host_prep(w1, b1, w2, b2, wg, bg):
    w1p = (np.asarray(w1, np.float32).T / 1764.0)
    w1aug = np.concatenate([w1p, np.asarray(b1, np.float32)[None, :]], 0)

    # kern rows (g, slot): w2r/b2r ordered so slot s corresponds to hw
    # SLOT_HW[s]; w2 columns (i) follow s1's hw->slot ordering via pooled.
    w2v = np.asarray(w2, np.float32).reshape(R, COUT, CIN, R)
    w2r = w2v.transpose(0, 3, 2, 1).reshape(64, CIN * COUT)
    b2v = np.asarray(b2, np.float32).reshape(R, COUT, CIN)
    b2r = b2v.transpose(0, 2, 1).reshape(R, CIN * COUT)
    krhs = np.ascontiguousarray(
        np.concatenate([w2r, b2r], 0).astype(np.float16))

    sdelta = np.zeros((8, 72), np.float16)
    for g in range(8):
        sdelta[g, 9 * g: 9 * g + 9] = 1.0

    wgv = np.asarray(wg, np.float32)          # [8, 64, 3, 3]
    wg16 = wgv.astype(np.float16)
    wgl = wgv - wg16.astype(np.float32)       # fp16 residual

    def tapw(a, hw):
        dy, dx = hw // 3, hw % 3
        return a[:, :, dy, dx].T              # [cin, 8]

    # guide main fp16: 5 slots (pairs + tap8 single)
    wgp = np.zeros((128, 5, 8), np.float16)
    for s in range(4):
        wgp[0:64, s, :] = tapw(wg16, SLOT_HW[2 * s])
        wgp[64:128, s, :] = tapw(wg16, SLOT_HW[2 * s + 1])
    wgp[0:64, 4, :] = tapw(wg16, 8)

    # guide corr DR lhsT [128, mm, ktile, 8]:
    # mm0 ktiles (dy0, dy1): taps (0,1), (3,4)
    # mm1 ktiles (dy2 dx0, dy2 dx2): taps (6,7), (8, zero)
    # mm2 ktiles (dy0 dx2, dy1 dx2): taps (2, zero), (5, zero)
    def corr_pack(a, dtype):
        p = np.zeros((128, 3, 2, 16), np.float32)
        p[0:64, 0, 0, 0:8] = tapw(a, 0)
        p[64:128, 0, 0, 0:8] = tapw(a, 1)
        p[0:64, 0, 1, 0:8] = tapw(a, 3)
        p[64:128, 0, 1, 0:8] = tapw(a, 4)
        p[0:64, 1, 0, 0:8] = tapw(a, 6)
        p[64:128, 1, 0, 0:8] = tapw(a, 7)
        p[0:64, 1, 1, 0:8] = tapw(a, 8)
        p[0:64, 2, 0, 0:8] = tapw(a, 2)
        p[0:64, 2, 1, 0:8] = tapw(a, 5)
        return p.astype(dtype)

    wl8 = corr_pack(wgl * 256.0, ml_dtypes.float8_e5m2)
    wh8 = corr_pack(wg16.astype(np.float32), ml_dtypes.float8_e4m3)

    bg8 = np.asarray(bg, np.float32).reshape(8, 1)

    ec16 = np.zeros((8, 4, 128), np.float16)
    for c in range(4):
        ec16[2 * c, c, 0:64] = BIG
        ec16[2 * c + 1, c, 64:128] = BIG

    ones64 = np.zeros((128, 64), np.float16)
    for k in range(128):
        ones64[k, k % 64] = 1.0

    return dict(w1aug=np.ascontiguousarray(w1aug), krhs=krhs, sdelta=sdelta,
                wgp16=wgp, wl8=wl8, wh8=wh8, bg8=bg8, ec16=ec16,
                ones64=ones64)


def shard_x(xb):
    """One sample [64, 126, 126] -> 4 shifted SBUF layouts."""
    xf = np.ascontiguousarray(np.asarray(xb, np.float32).reshape(CIN, HW2))
    x16 = xf.astype(np.float16)
    x8 = xf.astype(ml_dtypes.float8_e4m3)
    gl = ((xf - x16.astype(np.float32)) * 256.0).astype(ml_dtypes.float8_e5m2)

    def stack(a, shift, dtype):
        t = np.zeros((128, HW2), dtype)
        t[0:64] = a
        t[64:128, 0:HW2 - shift] = a[:, shift:]
        return t

    return {
        "xa16": stack(x16, 1, np.float16),
        "x216": stack(x16, 126, np.float16),
        "xa8": stack(x8, 1, ml_dtypes.float8_e4m3),
        "gxa8": stack(gl, 1, ml_dtypes.float8_e5m2),
    }


_NC_CACHE = {}


def kernel(x, w1, b1, w2, b2, wg, bg, _profile=None):
    x = np.asarray(x, np.float32)
    Bn = x.shape[0]
    assert Bn == 8
    weights = host_prep(w1, b1, w2, b2, wg, bg)

    if "nc" not in _NC_CACHE:
        _NC_CACHE["nc"] = build_module()
    nc = _NC_CACHE["nc"]

    in_maps = []
    for b in range(Bn):
        m = shard_x(x[b])
        m.update(weights)
        in_maps.append(m)

    kwargs = dict(_profile.get("kwargs", {})) if _profile else {}
    res = run_bass_kernel_spmd(nc, in_maps, core_ids=list(range(Bn)), **kwargs)
    if _profile is not None:
        _profile["result"] = res

    out = np.stack([res.results[b]["y"].reshape(COUT, HO, WO)
                    for b in range(Bn)])
    return out.astype(np.float32)



# revision 29
# speedup vs baseline: 1.1847x; 1.1847x over previous
"""DRConv2d Trainium2 kernel — batch-parallel over 8 NeuronCores.

Per core (one sample b): x_b [64, 126, 126] -> out_b [64, 124, 124]

Math (per sample):
  pooled = avgpool3x3(x); s1 = sigmoid(w1 @ pooled + b1)
  kern[g] = w2[g] @ s1[g] + b2[g]          -> per-sample filter bank [8*64, 64, 3, 3]
  out_r = conv3x3_valid(x, kern)           -> [8, 64, 124, 124]
  guide = conv3x3_valid(x, wg) + bg        -> [8, 124, 124]
  out = out_{argmax_r guide}               -> [64, 124, 124]

Device strategy (cost-model-aware: matmul cost = out_free_size x dtype_rate,
independent of K and M; fp8 DoubleRow = 0.5x):
  - main conv fp16: x as two partition-stacked shifted layouts
    xa16=[v@0;v@+1], x216=[v@0;v@+126]; 9-tap conv = 4 paired K=128 matmuls
    per 128-channel chunk + 1 combined matmul (see below).
  - guide conv: fp16 main pass (5 matmuls) + compensated corrections
    (wl@x + wh@xl) in fp8 DoubleRow (6 matmuls at 0.5 = 3 slots), keeping
    argmax flips at the fp16x2-compensated level.
  - region one-hot mask: gpsimd partition all-reduce max + DVE is_equal.
  - selection: BIG*mask injected into each conv psum chunk through the
    combined matmul (tap-8 weights rows 0:64 + BIG*E_c rows 64:72, rhs is a
    staged [72,TN] tile holding the tap-8 x window + mask rows), then
    u_c = relu(P_c - BIG/2) and a DVE add-tree; no emask/fold matmuls.
  - one interleaved loop over 31 position tiles (4 output rows each).
"""
import numpy as np
import ml_dtypes

import concourse.bass as bass
import concourse.mybir as mybir
import concourse.tile as tile
from concourse import bacc, bass_isa, library_config
from concourse.bass_utils import run_bass_kernel_spmd

F32 = mybir.dt.float32
F16 = mybir.dt.float16
FP8 = mybir.dt.float8e4
FP8E5 = mybir.dt.float8e5
AL = mybir.AluOpType
AFT = mybir.ActivationFunctionType
DR = mybir.MatmulPerfMode.DoubleRow

R, CIN, COUT = 8, 64, 64
H, W = 126, 126
HO, WO = 124, 124
NPOS = HO * WO          # 15376
HW2 = H * W             # 15876
NT = 31                 # position tiles, 4 output rows each
TN = 4 * WO             # 496 positions per tile
G = 16                  # guide tiles run ahead of conv
GPRE = 16               # guide tiles before the generator block
BIG = 64.0              # mask inject magnitude; relu offset is BIG/2
CSC = 2.0 ** -8         # guide correction scale (host pre-scales by 2^8)

# conv/guide tap pairing: kern rows are (g, slot), pair taps at adjacent
# slots: slot -> hw: (0,1),(3,4),(6,7),(2,5) pairs + tap8 at slot 8
SLOT_HW = (0, 1, 3, 4, 6, 7, 2, 5, 8)
CPAIRS = [("xa16", 0, 0), ("xa16", 1, 0), ("xa16", 2, 0), ("x216", 0, 2)]
GMAIN = [("xa16", 0, 0, 128), ("xa16", 1, 0, 128), ("xa16", 2, 0, 128),
         ("x216", 0, 2, 128), ("x216", 2, 2, 64)]
# guide corr DR rhs windows: (ktile_stride, dy, dx)
CORR_SPECS = [(W, 0, 0), (2, 2, 0), (W, 0, 2)]


def _win(xv, t, dy, dx, k=128):
    r0 = 4 * t
    return xv[0:k, r0 + dy: r0 + dy + 4, dx: dx + WO]


def _corr_ap(xt, t, ktile_stride, dy, dx):
    """Manual DR rhs AP: [partition 128][ktile 2][row 4][col 124] with
    overlapping strides on the flat [128, HW2] x tile."""
    base = xt[:]
    off = base.offset + (4 * t + dy) * W + dx
    return bass.AP(base.tensor, off, [[HW2, 128], [ktile_stride, 2],
                                      [W, 4], [1, WO]])


def build_module():
    nc = bacc.Bacc(trn_type="TRN2", target_bir_lowering=False, debug=False,
                   num_devices=8)

    d_xs = {}
    for n, dt in (("xa16", F16), ("x216", F16), ("xa8", FP8), ("gxa8", FP8E5)):
        d_xs[n] = nc.dram_tensor(n, (128, HW2), dt, kind="ExternalInput")
    d_w1aug = nc.dram_tensor("w1aug", (65, 64), F32, kind="ExternalInput")
    d_krhs = nc.dram_tensor("krhs", (72, 4096), F16, kind="ExternalInput")
    d_sdelta = nc.dram_tensor("sdelta", (8, 72), F16, kind="ExternalInput")
    d_wgp16 = nc.dram_tensor("wgp16", (128, 5, 8), F16, kind="ExternalInput")
    d_wl = nc.dram_tensor("wl8", (128, 3, 2, 16), FP8E5, kind="ExternalInput")
    d_wh = nc.dram_tensor("wh8", (128, 3, 2, 16), FP8, kind="ExternalInput")
    d_bg = nc.dram_tensor("bg8", (8, 1), F32, kind="ExternalInput")
    d_ec = nc.dram_tensor("ec16", (8, 4, 128), F16, kind="ExternalInput")
    d_y = nc.dram_tensor("y", (COUT, NPOS), F16, kind="ExternalOutput")

    with tile.TileContext(nc) as tc:
        kernel_body(nc, tc, d_xs, d_w1aug, d_krhs, d_sdelta, d_wgp16,
                    d_wl, d_wh, d_bg, d_ec, d_y)
    nc.compile()
    return nc


def kernel_body(nc, tc, d_xs, d_w1aug, d_krhs, d_sdelta, d_wgp16,
                d_wl, d_wh, d_bg, d_ec, d_y):
    nc.gpsimd.load_library(library_config.mlp)

    with (
        tc.tile_pool(name="sbx", bufs=1) as sbx,
        tc.tile_pool(name="sbw", bufs=1) as sbw,
        tc.tile_pool(name="sbk", bufs=1) as sbk,
        tc.tile_pool(name="sbg", bufs=3) as sbg,
        tc.tile_pool(name="sst", bufs=G + 3) as sst,
        tc.tile_pool(name="sbu", bufs=2) as sbu,
        tc.tile_pool(name="sbo", bufs=3) as sbo,
        tc.tile_pool(name="p1", bufs=4, space="PSUM") as p1,   # conv chunks
        tc.tile_pool(name="p2", bufs=3, space="PSUM") as p2,   # guide
        tc.tile_pool(name="pm", bufs=1, space="PSUM") as pm,   # fold upper half
    ):
        # ---- small weights on scalar ring; bulk on sync/scalar/pool ----
        wgp16 = sbw.tile([128, 5, 8], F16, tag="wgp16")
        nc.scalar.dma_start(wgp16[:], d_wgp16.ap())
        wl8, wh8 = [], []
        for m in range(3):
            tl = sbw.tile([128, 2, 16], FP8E5, tag=f"wl8_{m}")
            nc.scalar.dma_start(tl[:], d_wl.ap()[:, m, :, :])
            wl8.append(tl)
            th = sbw.tile([128, 2, 16], FP8, tag=f"wh8_{m}")
            nc.scalar.dma_start(th[:], d_wh.ap()[:, m, :, :])
            wh8.append(th)
        bg8 = sbw.tile([8, 1], F32, tag="bg8")
        nc.scalar.dma_start(bg8[:], d_bg.ap())

        krhs = sbk.tile([72, 4096], F16, tag="krhs")
        nc.sync.dma_start(krhs[:], d_krhs.ap())
        w1aug = sbw.tile([65, 64], F32, tag="w1aug")
        nc.sync.dma_start(w1aug[:], d_w1aug.ap())
        sdelta_t = sbw.tile([8, 72], F16, tag="sdelta")
        nc.sync.dma_start(sdelta_t[:], d_sdelta.ap())

        nb32 = sbw.tile([128, 1], F32, tag="nb32")
        nc.vector.memset(nb32[:], -BIG / 2)

        # x layouts, quarter-granular DMAs (fp16 on sync/scalar, fp8 on pool)
        xt = {}
        NQ = 4
        Q = (HW2 + NQ - 1) // NQ
        for n, dt in (("xa16", F16), ("x216", F16), ("xa8", FP8),
                      ("gxa8", FP8E5)):
            xt[n] = sbx.tile([128, HW2], dt, tag=n, name=n)
        for qi in range(NQ):
            lo, hi = Q * qi, min(Q * (qi + 1), HW2)
            nc.sync.dma_start(xt["xa16"][:, lo:hi], d_xs["xa16"].ap()[:, lo:hi])
            nc.sync.dma_start(xt["x216"][:, lo:hi],
                              d_xs["x216"].ap()[:, lo:hi])
            nc.sync.dma_start(xt["xa8"][:, lo:hi], d_xs["xa8"].ap()[:, lo:hi])
            nc.gpsimd.dma_start(xt["gxa8"][:, lo:hi],
                                d_xs["gxa8"].ap()[:, lo:hi])
        xv16 = {n: xt[n][:].rearrange("p (h w) -> p h w", h=H)
                for n in ("xa16", "x216")}

        LW = sbw.tile([128, 16, 128], F16, tag="LW")
        LWE = sbw.tile([72, 4, 128], F16, tag="LWE")
        # mask-inject rows of LWE: BIG at (2c -> cols 0:64), (2c+1 -> 64:128)
        nc.scalar.dma_start(LWE[64:72, :, :], d_ec.ap())

        sts = {}

        def guide_tile(t):
            pg = p2.tile([40, TN], F32, tag="pg", name=f"pg{t}")
            for s, (xn, dy, dx, k) in enumerate(GMAIN):
                nc.tensor.matmul(pg[32:40, :], wgp16[0:k, s, :],
                                 _win(xv16[xn], t, dy, dx, k),
                                 start=(s == 0), stop=(s == 4),
                                 skip_group_check=True)
            # corr: pass A (wl*2^8 e5m2 @ xa8) + pass B (wh e4m3 @ gxa8)
            # mm0: ktiles dy0,dy1 -> taps 0,1,3,4
            # mm1: ktiles (dy2,dx0),(dy2,dx2) -> taps 6,7 + 8 (half zeroed)
            # mm2: ktiles (dy0,dx2),(dy1,dx2) -> taps 2,5 on top halves
            i = 0
            for wt, xn in ((wl8, "xa8"), (wh8, "gxa8")):
                for m, (ks, dy, dx) in enumerate(CORR_SPECS):
                    nc.tensor.matmul(pg[0:16, :], wt[m][:],
                                     _corr_ap(xt[xn], t, ks, dy, dx),
                                     start=(i == 0), stop=(i == 5),
                                     perf_mode=DR, skip_group_check=True)
                    i += 1
            gs = sbg.tile([8, TN], F32, tag="gs", name=f"gs{t}")
            nc.scalar.activation(gs[:], pg[32:40, :], AFT.Identity, bias=bg8[:])
            g = sbg.tile([8, TN], F32, tag="g", name=f"g{t}")
            nc.vector.scalar_tensor_tensor(g[:], pg[0:8, :], CSC, gs[:],
                                           op0=AL.mult, op1=AL.add)
            gm = sbg.tile([8, TN], F32, tag="gm", name=f"gm{t}")
            nc.gpsimd.partition_all_reduce(gm[:], g[:], channels=8,
                                           reduce_op=bass_isa.ReduceOp.max)
            st = sst.tile([72, TN], F16, tag="st", name=f"st{t}")
            stv = st[:].rearrange("p (r c) -> p r c", r=4)
            nc.gpsimd.tensor_copy(stv[0:64, :, :], _win(xv16["x216"], t, 2, 2, 64))
            nc.vector.tensor_tensor(st[64:72, :], g[:], gm[:], op=AL.is_equal)
            sts[t] = st

        pooled = sbg.tile([65, 9], F32, tag="pooled")
        nc.vector.memset(pooled[64:65, :], 1.0)

        # pooled block sums via ACT accumulator, sliced into load-idle ACT
        # windows: kh rows 0/1 after early guides, kh=2 after more guides.
        # pooled cols in slot order: (kh,kw<2) -> 2*kh+kw, (kh,2) -> 6+kh
        pscr = sbk.tile([64, 42, 42], F16, tag="pscr")

        def pooled_slice(kh):
            for kw in range(3):
                col = 2 * kh + kw if kw < 2 else 6 + kh
                blk = xv16["xa16"][0:64, 42 * kh: 42 * kh + 42,
                                   42 * kw: 42 * kw + 42]
                nc.scalar.activation(pscr[:], blk, AFT.Copy,
                                     accum_out=pooled[0:64, col: col + 1])

        for t in range(GPRE):
            guide_tile(t)
            if t == 7:
                pooled_slice(0)
            elif t == 10:
                pooled_slice(1)
            elif t == 13:
                pooled_slice(2)

        # ---- generator: s1 -> kern (fp16) -> LW/LWE via pool-ring DMAs ----
        s1p = p1.tile([64, 9], F32, tag="pc", name="s1p")
        nc.tensor.matmul(s1p[:], w1aug[:], pooled[:], start=True, stop=True)
        s1s = sbg.tile([64, 9], F16, tag="s1s")
        nc.scalar.activation(s1s[:], s1p[:], AFT.Sigmoid)

        S = sbk.tile([72, 72], F16, tag="S")
        nc.vector.memset(S[:], 0.0)
        nc.scalar.copy(S[64:72, :], sdelta_t[:])
        for gi in range(8):
            nc.sync.dma_start(S[8 * gi:8 * gi + 8, 9 * gi:9 * gi + 9],
                              s1s[8 * gi:8 * gi + 8, :])

        ksb = sbk.tile([72, 4096], F16, tag="ksb")
        for j in range(8):
            pk = p1.tile([72, 512], F32, tag="pc", name=f"pk{j}")
            nc.tensor.matmul(pk[:], S[:], krhs[:, 512 * j: 512 * (j + 1)],
                             start=True, stop=True)
            nc.scalar.copy(ksb[:, 512 * j: 512 * (j + 1)], pk[:])

        # LW[s*64+ci, c*4+p, h*64+co] <- ksb[g*9+2p+s, ci*64+co], g = 2c+h
        ksv = ksb[:].rearrange("p (ci co) -> p ci co", ci=64)
        ring = [nc.sync, nc.scalar, nc.gpsimd]
        ri = 0
        for c in range(4):
            for p in range(4):
                for hh in range(2):
                    gg = 2 * c + hh
                    ring[ri % 3].dma_start(
                        LW[0:128, c * 4 + p, 64 * hh:64 * hh + 64],
                        ksv[gg * 9 + 2 * p: gg * 9 + 2 * p + 2, :, :])
                    ri += 1
            for hh in range(2):
                gg = 2 * c + hh
                ring[ri % 3].dma_start(
                    LWE[0:64, c, 64 * hh:64 * hh + 64],
                    ksv[gg * 9 + 8: gg * 9 + 9, :, :])
                ri += 1

        for t in range(GPRE, G):
            guide_tile(t)

        # ---- main loop: guide(t+G) + conv(t), relu-select, add-tree ----
        otd = [None]

        def conv_tile(t):
            st = sts.pop(t)
            pcs = []
            for c in range(4):
                pc = p1.tile([128, TN], F32, tag="pc", name=f"pc{c}_{t}")
                for p, (xn, dy, dx) in enumerate(CPAIRS):
                    nc.tensor.matmul(pc[:], LW[0:128, 4 * c + p, :],
                                     _win(xv16[xn], t, dy, dx, 128),
                                     start=(p == 0), stop=False)
                nc.tensor.matmul(pc[:], LWE[0:72, c, :], st[:],
                                 start=False, stop=True)
                pcs.append(pc)
            us = []
            for c in range(4):
                u = sbu.tile([128, TN], F16, tag=f"u{c}", name=f"u{c}_{t}")
                nc.scalar.activation(u[:], pcs[c][:], AFT.Relu, bias=nb32[:])
                us.append(u)
            v0 = sbu.tile([128, TN], F16, tag="v0", name=f"v0_{t}")
            nc.vector.tensor_tensor(v0[:], us[0][:], us[1][:], op=AL.add)
            v1 = sbu.tile([128, TN], F16, tag="v1", name=f"v1_{t}")
            nc.vector.tensor_tensor(v1[:], us[2][:], us[3][:], op=AL.add)
            # fold 128->64 partitions + -BIG/2 bias on DVE (no matmul).
            # SB+SB operands must share a base partition, so route the upper
            # half through PSUM (SB+PSUM may differ in base partition).
            m0 = sbu.tile([64, TN], F16, tag="m0", name=f"m0_{t}")
            nc.vector.tensor_tensor(m0[:], v0[0:64, :], v1[0:64, :], op=AL.add)
            m1 = pm.tile([64, TN], F32, tag="m1", name=f"m1_{t}")
            nc.vector.tensor_tensor(m1[:], v0[64:128, :], v1[64:128, :],
                                    op=AL.add)
            if t % 2 == 0:
                otd[0] = sbo.tile([64, 2 * TN], F16, tag="otd", name=f"otd{t}")
            half = otd[0][:, (t % 2) * TN: (t % 2) * TN + TN]
            nc.vector.scalar_tensor_tensor(half, m0[:], -BIG / 2, m1[:],
                                           op0=AL.add, op1=AL.add)
            if t % 2 == 1:
                nc.sync.dma_start(d_y.ap()[:, TN * (t - 1): TN * (t + 1)],
                                  otd[0][:])
            elif t == NT - 1:
                nc.sync.dma_start(d_y.ap()[:, TN * t: TN * (t + 1)],
                                  otd[0][:, 0:TN])

        for t in range(NT):
            if t + G < NT:
                guide_tile(t + G)
            conv_tile(t)


def host_prep(w1, b1, w2, b2, wg, bg):
    w1p = (np.asarray(w1, np.float32).T / 1764.0)
    w1aug = np.concatenate([w1p, np.asarray(b1, np.float32)[None, :]], 0)

    # kern rows (g, slot): w2r/b2r ordered so slot s corresponds to hw
    # SLOT_HW[s]; w2 columns (i) follow s1's hw->slot ordering via pooled.
    w2v = np.asarray(w2, np.float32).reshape(R, COUT, CIN, R)
    w2r = w2v.transpose(0, 3, 2, 1).reshape(64, CIN * COUT)
    b2v = np.asarray(b2, np.float32).reshape(R, COUT, CIN)
    b2r = b2v.transpose(0, 2, 1).reshape(R, CIN * COUT)
    krhs = np.ascontiguousarray(
        np.concatenate([w2r, b2r], 0).astype(np.float16))

    sdelta = np.zeros((8, 72), np.float16)
    for g in range(8):
        sdelta[g, 9 * g: 9 * g + 9] = 1.0

    wgv = np.asarray(wg, np.float32)          # [8, 64, 3, 3]
    wg16 = wgv.astype(np.float16)
    wgl = wgv - wg16.astype(np.float32)       # fp16 residual

    def tapw(a, hw):
        dy, dx = hw // 3, hw % 3
        return a[:, :, dy, dx].T              # [cin, 8]

    # guide main fp16: 5 slots (pairs + tap8 single)
    wgp = np.zeros((128, 5, 8), np.float16)
    for s in range(4):
        wgp[0:64, s, :] = tapw(wg16, SLOT_HW[2 * s])
        wgp[64:128, s, :] = tapw(wg16, SLOT_HW[2 * s + 1])
    wgp[0:64, 4, :] = tapw(wg16, 8)

    # guide corr DR lhsT [128, mm, ktile, 8]:
    # mm0 ktiles (dy0, dy1): taps (0,1), (3,4)
    # mm1 ktiles (dy2 dx0, dy2 dx2): taps (6,7), (8, zero)
    # mm2 ktiles (dy0 dx2, dy1 dx2): taps (2, zero), (5, zero)
    def corr_pack(a, dtype):
        p = np.zeros((128, 3, 2, 16), np.float32)
        p[0:64, 0, 0, 0:8] = tapw(a, 0)
        p[64:128, 0, 0, 0:8] = tapw(a, 1)
        p[0:64, 0, 1, 0:8] = tapw(a, 3)
        p[64:128, 0, 1, 0:8] = tapw(a, 4)
        p[0:64, 1, 0, 0:8] = tapw(a, 6)
        p[64:128, 1, 0, 0:8] = tapw(a, 7)
        p[0:64, 1, 1, 0:8] = tapw(a, 8)
        p[0:64, 2, 0, 0:8] = tapw(a, 2)
        p[0:64, 2, 1, 0:8] = tapw(a, 5)
        return p.astype(dtype)

    wl8 = corr_pack(wgl * 256.0, ml_dtypes.float8_e5m2)
    wh8 = corr_pack(wg16.astype(np.float32), ml_dtypes.float8_e4m3)

    bg8 = np.asarray(bg, np.float32).reshape(8, 1)

    ec16 = np.zeros((8, 4, 128), np.float16)
    for c in range(4):
        ec16[2 * c, c, 0:64] = BIG
        ec16[2 * c + 1, c, 64:128] = BIG

    return dict(w1aug=np.ascontiguousarray(w1aug), krhs=krhs, sdelta=sdelta,
                wgp16=wgp, wl8=wl8, wh8=wh8, bg8=bg8, ec16=ec16)


def shard_x(xb):
    """One sample [64, 126, 126] -> 4 shifted SBUF layouts."""
    xf = np.ascontiguousarray(np.asarray(xb, np.float32).reshape(CIN, HW2))
    x16 = xf.astype(np.float16)
    x8 = xf.astype(ml_dtypes.float8_e4m3)
    gl = ((xf - x16.astype(np.float32)) * 256.0).astype(ml_dtypes.float8_e5m2)

    def stack(a, shift, dtype):
        t = np.zeros((128, HW2), dtype)
        t[0:64] = a
        t[64:128, 0:HW2 - shift] = a[:, shift:]
        return t

    return {
        "xa16": stack(x16, 1, np.float16),
        "x216": stack(x16, 126, np.float16),
        "xa8": stack(x8, 1, ml_dtypes.float8_e4m3),
        "gxa8": stack(gl, 1, ml_dtypes.float8_e5m2),
    }


_NC_CACHE = {}


def kernel(x, w1, b1, w2, b2, wg, bg, _profile=None):
    x = np.asarray(x, np.float32)
    Bn = x.shape[0]
    assert Bn == 8
    weights = host_prep(w1, b1, w2, b2, wg, bg)

    if "nc" not in _NC_CACHE:
        _NC_CACHE["nc"] = build_module()
    nc = _NC_CACHE["nc"]

    in_maps = []
    for b in range(Bn):
        m = shard_x(x[b])
        m.update(weights)
        in_maps.append(m)

    kwargs = dict(_profile.get("kwargs", {})) if _profile else {}
    res = run_bass_kernel_spmd(nc, in_maps, core_ids=list(range(Bn)), **kwargs)
    if _profile is not None:
        _profile["result"] = res

    out = np.stack([res.results[b]["y"].reshape(COUT, HO, WO)
                    for b in range(Bn)])
    return out.astype(np.float32)

